# revision 71
# baseline (speedup 1.0000x reference)
"""Trainium2 Bass kernel for the quantized ResNet Bottleneck block (v3).

Strategy
--------
Data parallel over batch: 64 images -> 8 cores x 8 images. Each core runs an
identical Bass program; weights are replicated.

All convs are integer-valued matmuls accumulated in fp32 PSUM (exact:
quantized codes are integers; code offsets pass through each conv as
per-output-channel constants folded into the next bias on the host).

Numeric scheme:
  * x is fake-quantized ON THE HOST: codes c0 = clip(rne(254*x), -127, 127)
    shipped PLAIN (no offset) as bf16 (ints <= 127 are exact). This removes
    the on-device x-quant DVE passes entirely and the shortcut conv needs
    no column-sum correction.
  * conv1/conv2 epilogues are ONE custom DVE op each (QEPI):
    out = clip(rne(a*psum + beta), 0, 127) + 128, rounding via the f32
    magic-add (v + 1.5*2^23) trick -- exact RNE on the integer grid.
    Outputs (codes+128 in [1,255]) are exact in bf16. Pad value for conv2
    is 128; the +128 offsets fold into the next layer's bias via weight
    column sums.
  * conv3 + stride-2 shortcut accumulate into one PSUM tile (shortcut
    weights pre-scaled by css/c3s on host); epilogue is ONE custom DVE op
    (FEPI): y = min(relu(g3*psum + delta), 6), bf16 out, per-m store.

Engine placement (learned from HW traces):
  * NO elementwise work on GpSimd: Q7 tensor ops starve SBUF arbitration
    for every other engine (DVE ops degrade 350ns -> 6us while a GpSimd
    tensor_scalar runs). GpSimd only does pad memsets + tiny SWDGE DMAs.
  * DVE runs all epilogues as fused custom ops (~0.55us per [128,392]).
  * ACT only builds per-channel bias broadcast tiles once at startup.
  * Weights ride the ACT HWDGE queue, x codes + y stores the SP queue, so
    the weight stream does not delay the x stream.
  * DVE APs kept strictly 2D-contiguous where possible (3-dim views drop
    DVE to 1 elem/cycle on HW).

Schedule: x ships with a host-duplicated ee-plane per k-chunk so the
stride-2 shortcut conv reads a contiguous rhs; conv1's padded output
lives in a column-parity layout so conv2's 9 taps read inner-contiguous
rhs (strided rhs costs ~30-40ns per 392-row matmul). Pair-0/1 x chunks
interleave across both HWDGE queues (SP + ACT) in consumption order —
one queue sustains only ~144 B/ns and conv1 consumes ~300. Weight
layouts are pre-transposed on the host (device-side rearranged DMAs pay
7-11us of descriptor generation). Software pipeline: conv2/conv3 of
pair p-1 run under conv1 of pair p; per-m output stores issue on the
ACT queue right after each FEPI. A short run of dummy matmuls on the
zero tile warms the PE p-state ramp while the first x DMA flies.
"""

import sys
from contextlib import ExitStack

import numpy as np

sys.path.insert(0, "/opt/trn_rl_repo")

import ml_dtypes  # noqa: E402

import concourse.bacc as bacc  # noqa: E402
import concourse.dve_ops as dve_ops  # noqa: E402
import concourse.tile as tile  # noqa: E402
from concourse import mybir  # noqa: E402
from concourse.bass_utils import run_bass_kernel_spmd  # noqa: E402
from concourse.dve_spec import (  # noqa: E402
    C0 as DC0,
    C1 as DC1,
    C2 as DC2,
    One as DOne,
    Spec,
    Src0 as DSrc0,
    Src1 as DSrc1,
    _has_src1,
    lower as dve_lower,
    maxx,
    minn,
    relu as drelu,
)
from concourse.dve_uop import DveOpSpec  # noqa: E402
from concourse.dve_table_gen import dve_ver_for  # noqa: E402
from concourse.dve_ops import DveOp  # noqa: E402

F32 = mybir.dt.float32
F16 = mybir.dt.float16
BF16 = mybir.dt.bfloat16
ALU = mybir.AluOpType
AFT = mybir.ActivationFunctionType
BF16NP = ml_dtypes.bfloat16

C_MAGIC = float(np.float32(12582912.0))  # 1.5 * 2**23

N_CORES = 8
B_LOC = 8  # images per core
PAIRS = B_LOC // 2


def _register_dve_op(name, spec, subdim=False):
    """Register a custom DVE op at runtime (table is generated per-NEFF)."""
    for o in dve_ops.OPS:
        if o.name == name:
            return o
    row = dve_ops._CUSTOM_DVE_ROW_BASE + len(dve_ops.OPS)
    assert row < 0x20
    shas = {}
    for ver in ("v3", "v4"):
        tmp = DveOpSpec(
            name=name, opcode=row, uops=dve_lower(spec, ver=ver),
            rd1_en=_has_src1(spec),
        )
        shas[ver] = tmp.sha(ver)
    op = DveOp(name, spec, subdim=subdim, uops_sha=shas)
    dve_ops.OPS.append(op)
    dve_ops._SUB_OPCODE_FOR_NAME[name] = row
    dve_ops.CUSTOM_DVE_SPECS[name] = spec
    return op


def _b(in0, in1):
    """Sim helper: in1 streams elementwise on HW; align shapes for numpy."""
    if isinstance(in1, np.ndarray) and in1.size == in0.size:
        return in1.reshape(in0.shape)
    return in1


# quantize epilogue: v = in0*s0 + in1 ; out = clip(rne(v),0,127) + 128
# rne via magic-add: u = v + C rounds to the integer grid (f32 ulp==1 in
# [2^23, 2^24)); clip in the shifted domain [C, C+127]; subtract C-128.
QEPI = _register_dve_op(
    "BNECK_QEPI_ANT",
    Spec(
        body=(minn(maxx((DSrc0 * DC0 + DSrc1) + DC1, DC1), DC1 + DC2) - DC1)
        + (DC2 + DOne),
        reference=lambda in0, in1, s0, s1, imm2: np.minimum(
            np.maximum(np.round(in0 * s0 + _b(in0, in1)), 0.0), imm2
        )
        + (imm2 + 1.0),
    ),
)


# final epilogue: out = min(relu(in0*s0 + in1), s1)
FEPI = _register_dve_op(
    "BNECK_FEPI_ANT",
    Spec(
        body=minn(drelu(DSrc0 * DC0 + DSrc1), DC1),
        reference=lambda in0, in1, s0, s1, imm2: np.minimum(
            np.maximum(in0 * s0 + _b(in0, in1), 0.0), s1
        ),
    ),
)


def _build_nc(pairs=PAIRS):
    nc = bacc.Bacc("TRN2", target_bir_lowering=False, debug=False)

    # x codes, plain int values in [-127,127], layout [pair, q, k, (i h)]
    # per (q,k): 1568 full-res codes (i*784+h) then 392 ee-plane codes
    # (i*196 + stride-2 pixels) so the stride-2 shortcut conv reads a
    # contiguous rhs (strided rhs costs ~30-40ns per matmul on the PE).
    x_d = nc.dram_tensor("x", [pairs, 128, 4, 1960], BF16,
                         kind="ExternalInput")
    # weights pre-transposed to partition-major ON THE HOST: device DMAs
    # are then straight 2D copies (128 big descriptors) — on-device
    # rearranges cost 7-11us of descriptor generation on the DGE
    w1_d = nc.dram_tensor("w1l", [128, 4, 256], BF16, kind="ExternalInput")
    w2_d = nc.dram_tensor("w2l", [128, 9, 2, 256], BF16, kind="ExternalInput")
    w3_d = nc.dram_tensor("w3l", [128, 2, 1024], BF16, kind="ExternalInput")
    ws_d = nc.dram_tensor("wsl", [128, 4, 1024], BF16, kind="ExternalInput")
    b1_d = nc.dram_tensor("beta1", [128, 2], F32, kind="ExternalInput")
    b2_d = nc.dram_tensor("beta2", [128, 2], F32, kind="ExternalInput")
    dl_d = nc.dram_tensor("delta", [128, 8], F32, kind="ExternalInput")
    # packed [pair, m-pair, q, (m2 i h)] to keep the out-DMA 2D; host unpacks
    y_d = nc.dram_tensor("y", [pairs, 4, 128, 784], BF16, kind="ExternalOutput")

    a1, a2, g3 = _SCALES

    with tile.TileContext(nc) as tc, ExitStack() as ctx:
        wp = ctx.enter_context(tc.tile_pool(name="w", bufs=1))
        xqp = ctx.enter_context(tc.tile_pool(name="xq", bufs=3))
        p2p = ctx.enter_context(tc.tile_pool(name="p2", bufs=2))
        t3p = ctx.enter_context(tc.tile_pool(name="t3", bufs=2))
        yop = ctx.enter_context(tc.tile_pool(name="yo", bufs=6))
        pc1 = ctx.enter_context(tc.tile_pool(name="pc1", bufs=3, space="PSUM"))
        pc2 = ctx.enter_context(tc.tile_pool(name="pc2", bufs=2, space="PSUM"))
        pc3 = ctx.enter_context(tc.tile_pool(name="pc3", bufs=3, space="PSUM"))

        # ---- weights + biases ----
        # startup critical path: w1 is the very first ACT-queue DMA while
        # pair-0's first x chunk is the first SP DMA — the two sequencers
        # issue in parallel (each issue costs ~0.65us of sequencer time,
        # so queue assignment matters as much as transfer time). Bias
        # broadcasts (ACT compute) then naturally delay the bulk-weight
        # transfers past the startup x-bandwidth crunch. x pairs + nothing
        # else on SP, y stores on ACT, tiny biases on the GpSimd SWDGE
        # queue.
        # w1 k-slices interleave with the x chunks across both queues in
        # joint consumption order: k0 leads SP, k1-2 lead ACT (so the ACT
        # queue's x-k1 transfer isn't stuck behind all of w1), k3 rides SP
        # after x-k2. (SWDGE delivery tried and measured +7us; whole-w1
        # ahead of x-k1 on ACT measured a 3.7us conv1 stall at k1.)
        w1t = wp.tile([128, 4, 256], BF16, tag="w1t")
        nc.sync.dma_start(w1t[:, 0:1], w1_d[:, 0:1])
        nc.scalar.dma_start(w1t[:, 1:3], w1_d[:, 1:3])

        beta1 = wp.tile([128, 2], F32, tag="beta1")
        nc.gpsimd.dma_start(beta1[:], b1_d[:])
        beta2 = wp.tile([128, 2], F32, tag="beta2")
        nc.gpsimd.dma_start(beta2[:], b2_d[:])
        delta = wp.tile([128, 8], F32, tag="delta")
        nc.gpsimd.dma_start(delta[:], dl_d[:])

        # zero tile: ACT broadcast source + PE warmup operands
        rz = wp.tile([128, 392], BF16, tag="rz")
        nc.vector.memset(rz[:], 0.0)
        # warm the PE p-state ramp with dummy matmuls on the zero tile
        # (~3us of continuous PE work; real data lands at ~9.5us)
        pz = pc1.tile([128, 392], F32, tag="ps1", name="pz")
        for _ in range(10):
            nc.tensor.matmul(pz[:], rz[:, :128], rz[:], start=True, stop=True)
        # preload the ACT function table while the first DMAs fly
        warm_act = wp.tile([128, 1], F32, tag="warm_act")
        nc.scalar.activation(warm_act[:], rz[:, :1], AFT.Relu, bias=0.0, scale=1.0)

        st = {}  # per-pair tiles: xq, p2, t3

        def xq_alloc(p):
            xq = xqp.tile([128, 4, 1960], BF16, tag="xq", name=f"xq_{p}")
            st[p] = {"xq": xq}
            return xq

        def xpiece(eng, p, ks, lo, hi):
            """Load x_d[p][:, ks, lo:hi] on the given queue engine."""
            xq = st[p]["xq"]
            eng.dma_start(xq[:, ks, lo:hi], x_d[p][:, ks, lo:hi])

        def emit_xload(p):
            xq = xq_alloc(p)
            nc.sync.dma_start(xq[:], x_d[p])

        def emit_conv1(p, mid_cb=None):
            xq = st[p]["xq"]
            p2 = []
            for m in range(2):
                # p2 holds conv1 output codes+128 in COLUMN-PARITY layout
                # [pc, i, r(30), x(16)]: plane pc holds padded columns
                # c = 2x+pc, so conv2's stride-2 taps read inner-contiguous
                # rhs; rows padded to 30 PER IMAGE so one rank-3 slice
                # ky:ky+58:2 covers both images (29 rows incl 1 discarded
                # junk row) and conv2 runs N=406 matmuls instead of rank-4
                # rhs. pad value 128 == quantized zero; padded row 0 (both
                # images), the junk row 29, and padded column c=1 (pc=1,
                # x=0) get memsets.
                t = p2p.tile([128, 1856], BF16, tag=f"p2_{m}")
                pv = t.rearrange(
                    "q (pc i r x) -> q pc i r x", pc=2, i=2, r=29, x=16
                )
                nc.gpsimd.memset(pv[:, :, :, 0, :], 128.0)
                nc.gpsimd.memset(pv[:, 1, :, :, 0], 128.0)
                p2.append(t)
            for i in (0, 1):
                ps = {}
                # k-outer across the four (m, hf) quads; three PSUM tiles
                # from pc1, the fourth borrows a pc3 bank
                quads = [(0, 0), (0, 1), (1, 0), (1, 1)]
                for k in range(4):
                    for m, hf in quads:
                        if k == 0:
                            pool = pc3 if (m, hf) == (1, 1) else pc1
                            ps[(m, hf)] = pool.tile(
                                [128, 392], F32,
                                name=f"ps1_{m}_{hf}",
                                tag="ps3" if (m, hf) == (1, 1) else "ps1",
                            )
                        nc.tensor.matmul(
                            ps[(m, hf)][:],
                            w1t[:, k, m * 128 : (m + 1) * 128],
                            xq[:, k, i * 784 + hf * 392 : i * 784 + hf * 392 + 392],
                            start=(k == 0),
                            stop=(k == 3),
                            skip_group_check=True,
                        )
                for m in (0, 1):
                    pv = p2[m].rearrange(
                        "q (pc i r x) -> q pc i r x", pc=2, i=2, r=29, x=16
                    )
                    for hf in (0, 1):
                        psv = ps[(m, hf)].rearrange("q (j e) -> q j e", j=14)
                        for pc in (0, 1):
                            # fused epilogue: codes+128 via exact magic-add
                            # RNE; output cols c=2+e land at (pc=c&1, x=c//2)
                            nc.vector._custom_dve(
                                QEPI,
                                out=pv[
                                    :, pc, i, 1 + 14 * hf : 15 + 14 * hf, 1:15
                                ],
                                in0=psv[:, :, pc::2],
                                in1=bc1[m][:, :196],
                                s0=a1, s1=C_MAGIC, imm2=127.0,
                            )
                if i == 0 and mid_cb is not None:
                    mid_cb()
            st[p]["p2"] = p2

        def emit_conv2(p):
            p2 = st[p]["p2"]
            t3 = []
            for m in range(2):
                ps2 = pc2.tile([128, 392], F32)
                first = True
                for k in range(2):
                    pv = p2[k].rearrange(
                        "q (pc i r x) -> q pc i r x", pc=2, i=2, r=29, x=16
                    )
                    for tp in range(9):
                        ky, kx = divmod(tp, 3)
                        # tap (ky,kx) reads padded cols c=1+kx+2ox -> plane
                        # pc=(1+kx)&1, x=(1+kx)//2 .. +14; rows ky..+26
                        # step 2 (inner dim contiguous)
                        nc.tensor.matmul(
                            ps2[:],
                            w2t[:, tp, k, m * 128 : (m + 1) * 128],
                            pv[
                                :, (1 + kx) & 1, :, ky : min(ky + 28, 29) : 2,
                                (1 + kx) // 2 : (1 + kx) // 2 + 14,
                            ],
                            start=first,
                            stop=(k == 1 and tp == 8),
                        )
                        first = False
                t3m = t3p.tile([128, 392], BF16, tag=f"t3_{m}")
                nc.vector._custom_dve(
                    QEPI, out=t3m[:], in0=ps2[:], in1=bc2[m][:],
                    s0=a2, s1=C_MAGIC, imm2=127.0,
                )
                t3.append(t3m)
            st[p]["t3"] = t3

        def emit_conv3(p, last=False):
            xq, t3 = st[p]["xq"], st[p]["t3"]
            for m in range(8):
                ps3 = pc3.tile([128, 392], F32)
                # shortcut first: only needs xq (contiguous host-shipped
                # ee plane), giving t3's epilogue time
                for k in range(4):
                    nc.tensor.matmul(
                        ps3[:],
                        wst[:, k, m * 128 : (m + 1) * 128],
                        xq[:, k, 1568:1960],
                        start=(k == 0),
                        stop=False,
                        skip_group_check=True,
                    )
                for k in range(2):
                    nc.tensor.matmul(
                        ps3[:],
                        w3t[:, k, m * 128 : (m + 1) * 128],
                        t3[k][:],
                        start=False,
                        stop=(k == 1),
                        skip_group_check=True,
                    )
                # single fused DVE epilogue + per-m store keeps the chain
                # after each m's last matmul short and GpSimd-free
                yo = yop.tile([128, 392], BF16, tag="yot", name="yot")
                nc.vector._custom_dve(
                    FEPI, out=yo[:], in0=ps3[:], in1=dlf[m][:],
                    s0=g3, s1=6.0,
                )
                # last pair: alternate store queues so the final store's
                # issue isn't serialized behind the previous one
                seng = nc.sync if (last and m % 2 == 1) else nc.scalar
                seng.dma_start(
                    y_d[p, m // 2][:, (m % 2) * 392 : (m % 2) * 392 + 392],
                    yo[:],
                )
            del st[p]

        # pair 0+1 x chunks interleaved across BOTH HWDGE queues (one queue
        # sustains ~144 B/ns; conv1 eats ~300). conv1 consumes k in the
        # order 0,2,1,3 to match arrival (SP chunks land before ACT ones).
        xq_alloc(0)
        xq_alloc(1)
        xpiece(nc.sync, 0, 0, 0, 784)       # SP:  p0 k0 i0
        xpiece(nc.scalar, 0, 1, 0, 784)     # ACT: p0 k1 i0
        xpiece(nc.sync, 0, 2, 0, 784)       # SP:  p0 k2 i0
        nc.sync.dma_start(w1t[:, 3:4], w1_d[:, 3:4])  # SP: w1 k3
        xpiece(nc.scalar, 0, 3, 0, 784)     # ACT: p0 k3 i0
        bc1, bc2, dlf = [], [], []
        for m in range(2):
            t = wp.tile([128, 392], F32, tag=f"bc1_{m}", name=f"bc1_{m}")
            nc.scalar.activation(
                t[:], rz[:], AFT.Identity, bias=beta1[:, m : m + 1], scale=0.0
            )
            bc1.append(t)
        # p0's i1 block k-granular so conv1 i1 can start on k0's arrival
        # instead of a single 400KB completion event
        xpiece(nc.sync, 0, 0, 784, 1568)      # SP:  p0 i1 k0
        xpiece(nc.scalar, 0, 1, 784, 1568)    # ACT: p0 i1 k1
        xpiece(nc.sync, 0, 2, 784, 1568)      # SP:  p0 i1 k2
        xpiece(nc.scalar, 0, 3, 784, 1568)    # ACT: p0 i1 k3
        xpiece(nc.sync, 1, slice(0, 2), 0, 784)       # SP:  p1 i0 k01
        xpiece(nc.scalar, 1, slice(2, 4), 0, 784)     # ACT: p1 i0 k23
        xpiece(nc.sync, 1, slice(0, 2), 784, 1568)    # SP:  p1 i1 k01
        xpiece(nc.scalar, 1, slice(2, 4), 784, 1568)  # ACT: p1 i1 k23
        w2t = wp.tile([128, 9, 2, 256], BF16, tag="w2t")
        nc.scalar.dma_start(w2t[:], w2_d[:])
        for m in range(2):
            t = wp.tile([128, 392], F32, tag=f"bc2_{m}", name=f"bc2_{m}")
            nc.scalar.activation(
                t[:], rz[:], AFT.Identity, bias=beta2[:, m : m + 1], scale=0.0
            )
            bc2.append(t)
        xpiece(nc.sync, 0, slice(0, 4), 1568, 1960)   # SP: p0 ee planes
        xpiece(nc.sync, 1, slice(0, 4), 1568, 1960)   # SP: p1 ee planes
        wst = wp.tile([128, 4, 1024], BF16, tag="wst")
        nc.scalar.dma_start(wst[:], ws_d[:])
        w3t = wp.tile([128, 2, 1024], BF16, tag="w3t")
        nc.scalar.dma_start(w3t[:], w3_d[:])
        for j in range(8):
            t = wp.tile([128, 392], F32, tag=f"dlf{j}", name=f"dlf{j}")
            nc.scalar.activation(
                t[:], rz[:], AFT.Identity, bias=delta[:, j : j + 1], scale=0.0
            )
            dlf.append(t)

        emit_conv1(0)

        # software pipeline: conv2/conv3 of pair p-1 run under conv1 of pair p
        for p in range(1, pairs):
            emit_conv1(p)
            emit_conv2(p - 1)
            if p + 1 < pairs:
                emit_xload(p + 1)
            emit_conv3(p - 1)
        emit_conv2(pairs - 1)
        emit_conv3(pairs - 1, last=True)
    return nc


_SCALES = (1.0, 1.0, 1.0)


def _prep(w1, b1, w2, b2, w3, b3, wsw, bs):
    """Host-side weight quantization + constant folding (all tiny tensors)."""
    f32 = np.float32

    def qw(w):
        s = f32(np.max(np.abs(w)))
        wq = np.round(np.clip(w / s, f32(-1.0), f32(1.0)) * f32(127.0)).astype(
            np.float32
        )
        return wq, s

    def qb(b):
        return np.round(b * f32(127.0)).astype(np.float32)

    w1q, c1s = qw(w1)  # [256,512,1,1]
    w2q, c2s = qw(w2)  # [256,256,3,3]
    w3q, c3s = qw(w3)  # [1024,256,1,1]
    wsq, css = qw(wsw)  # [1024,512,1,1]
    B1, B2, B3, Bs = qb(b1), qb(b2), qb(b3), qb(bs)

    a1 = f32(2.0) * c1s / f32(127.0)
    a2 = f32(2.0) * c2s / f32(127.0)
    g3 = c3s / f32(2.0 * 16129.0)
    rho = css / c3s

    # lhsT layouts, partition(q)-major so device DMAs are plain 2D copies
    w1l = np.ascontiguousarray(
        w1q[:, :, 0, 0].T.reshape(4, 128, 256).transpose(1, 0, 2).astype(BF16NP)
    )  # [128, 4, 256]
    # w2 taps: [ky,kx] -> lhsT [cin, cout] per tap; -> [128, 9, 2, 256]
    w2l = np.ascontiguousarray(
        w2q.transpose(2, 3, 1, 0)
        .reshape(9, 2, 128, 256)
        .transpose(2, 0, 1, 3)
        .astype(BF16NP)
    )
    w3l = np.ascontiguousarray(
        w3q[:, :, 0, 0].T.reshape(2, 128, 1024).transpose(1, 0, 2).astype(BF16NP)
    )  # [128, 2, 1024]
    ws_sc = (rho * wsq[:, :, 0, 0]).astype(BF16NP)  # [1024,512] scaled bf16
    wsl = np.ascontiguousarray(
        ws_sc.T.reshape(4, 128, 1024).transpose(1, 0, 2)
    )  # [128, 4, 1024]

    # column sums for the +128 code-offset corrections (fp64 exact)
    K2 = w2q.astype(np.float64).sum(axis=(1, 2, 3))  # [256]
    K3 = w3q[:, :, 0, 0].astype(np.float64).sum(axis=1)  # [1024]

    # x codes ship plain (no offset); conv1/conv2 outputs carry +128
    beta1 = (f32(4.0) * B1).astype(np.float32)
    beta2 = (f32(4.0) * B2 - a2 * f32(128.0) * K2.astype(np.float32)).astype(
        np.float32
    )
    delta0 = B3 * c3s / (f32(127.0) * c2s) + Bs / f32(127.0)
    delta = (
        delta0 - (g3.astype(np.float64) * (128.0 * K3)).astype(np.float32)
    ).astype(np.float32)

    beta1 = np.ascontiguousarray(beta1.reshape(2, 128).T)  # [128,2]
    beta2 = np.ascontiguousarray(beta2.reshape(2, 128).T)
    delta = np.ascontiguousarray(delta.reshape(8, 128).T)  # [128,8]

    return dict(
        w1l=w1l, w2l=w2l, w3l=w3l, wsl=wsl,
        beta1=beta1, beta2=beta2, delta=delta,
        a1=float(a1), a2=float(a2), g3=float(g3),
    )


def _quant_x(x):
    """Host fake-quant of x: codes = clip(rne(x/0.5 clipped * 127)) as bf16,
    laid out [core, pair, k, q, (i h)]."""
    f32 = np.float32
    c = np.round(
        np.clip(x.astype(np.float32) / f32(0.5), f32(-1.0), f32(1.0)) * f32(127.0)
    )
    # [64, 512, 28, 28] -> [cores, pairs, 2(i), 4(k), 128(q), 28, 28]
    c = c.reshape(N_CORES, PAIRS, 2, 4, 128, 28, 28)
    # full-res block: [cores, pairs, q, k, i, 784]
    full = c.transpose(0, 1, 4, 3, 2, 5, 6).reshape(
        N_CORES, PAIRS, 128, 4, 1568
    )
    # ee plane (stride-2 pixels) for the shortcut conv: [.., q, k, i*196]
    ee = np.ascontiguousarray(c[..., 0::2, 0::2]).transpose(
        0, 1, 4, 3, 2, 5, 6
    ).reshape(N_CORES, PAIRS, 128, 4, 392)
    out = np.concatenate([full, ee], axis=-1)  # [.., 128, 4, 1960]
    return np.ascontiguousarray(out.astype(BF16NP))


def kernel(x, w1, b1, w2, b2, w3, b3, ws, bs):
    global _SCALES
    xc = _quant_x(np.asarray(x, np.float32))
    pre = _prep(
        np.asarray(w1, np.float32), np.asarray(b1, np.float32),
        np.asarray(w2, np.float32), np.asarray(b2, np.float32),
        np.asarray(w3, np.float32), np.asarray(b3, np.float32),
        np.asarray(ws, np.float32), np.asarray(bs, np.float32),
    )
    _SCALES = (pre["a1"], pre["a2"], pre["g3"])
    nc = _build_nc()
    nc.compile()

    shared = {
        "w1l": pre["w1l"], "w2l": pre["w2l"], "w3l": pre["w3l"],
        "wsl": pre["wsl"], "beta1": pre["beta1"], "beta2": pre["beta2"],
        "delta": pre["delta"],
    }
    in_maps = [{"x": xc[c], **shared} for c in range(N_CORES)]

    import os

    tmpdir = os.environ.get("KERNEL_TRACE_DIR") or None
    if tmpdir:
        os.makedirs(tmpdir, exist_ok=True)
    res = run_bass_kernel_spmd(nc, in_maps, list(range(N_CORES)), tmpdir=tmpdir)
    global LAST_RESULT
    LAST_RESULT = res
    outs = [unpack_y(res.results[c]["y"]) for c in range(N_CORES)]
    return np.ascontiguousarray(np.concatenate(outs, axis=0))


def unpack_y(y):
    """[pairs,4,128,784] packed -> [2*pairs, 1024, 14, 14]."""
    p = y.shape[0]
    y = y.reshape(p, 4, 128, 2, 2, 196)  # (p, mp, q, m2, i, h)
    y = y.transpose(0, 4, 1, 3, 2, 5)  # (p, i, mp, m2, q, h)
    return np.ascontiguousarray(
        y.reshape(2 * p, 1024, 14, 14).astype(np.float32)
    )


# revision 72
# speedup vs baseline: 1.0208x; 1.0208x over previous
"""Trainium2 Bass kernel for the quantized ResNet Bottleneck block (v3).

Strategy
--------
Data parallel over batch: 64 images -> 8 cores x 8 images. Each core runs an
identical Bass program; weights are replicated.

All convs are integer-valued matmuls accumulated in fp32 PSUM (exact:
quantized codes are integers; code offsets pass through each conv as
per-output-channel constants folded into the next bias on the host).

Numeric scheme:
  * x is fake-quantized ON THE HOST: codes c0 = clip(rne(254*x), -127, 127)
    shipped PLAIN (no offset) as bf16 (ints <= 127 are exact). This removes
    the on-device x-quant DVE passes entirely and the shortcut conv needs
    no column-sum correction.
  * conv1/conv2 epilogues are ONE custom DVE op each (QEPI):
    out = clip(rne(a*psum + beta), 0, 127) + 128, rounding via the f32
    magic-add (v + 1.5*2^23) trick -- exact RNE on the integer grid.
    Outputs (codes+128 in [1,255]) are exact in bf16. Pad value for conv2
    is 128; the +128 offsets fold into the next layer's bias via weight
    column sums.
  * conv3 + stride-2 shortcut accumulate into one PSUM tile (shortcut
    weights pre-scaled by css/c3s on host); epilogue is ONE custom DVE op
    (FEPI): y = min(relu(g3*psum + delta), 6), bf16 out, per-m store.

Engine placement (learned from HW traces):
  * NO elementwise work on GpSimd: Q7 tensor ops starve SBUF arbitration
    for every other engine (DVE ops degrade 350ns -> 6us while a GpSimd
    tensor_scalar runs). GpSimd only does pad memsets + tiny SWDGE DMAs.
  * DVE runs all epilogues as fused custom ops (~0.55us per [128,392]).
  * ACT only builds per-channel bias broadcast tiles once at startup.
  * Weights ride the ACT HWDGE queue, x codes + y stores the SP queue, so
    the weight stream does not delay the x stream.
  * DVE APs kept strictly 2D-contiguous where possible (3-dim views drop
    DVE to 1 elem/cycle on HW).

Schedule: x ships with a host-duplicated ee-plane per k-chunk so the
stride-2 shortcut conv reads a contiguous rhs; conv1's padded output
lives in a column-parity layout so conv2's 9 taps read inner-contiguous
rhs (strided rhs costs ~30-40ns per 392-row matmul). Pair-0/1 x chunks
interleave across both HWDGE queues (SP + ACT) in consumption order —
one queue sustains only ~144 B/ns and conv1 consumes ~300. Weight
layouts are pre-transposed on the host (device-side rearranged DMAs pay
7-11us of descriptor generation). Software pipeline: conv2/conv3 of
pair p-1 run under conv1 of pair p; per-m output stores issue on the
ACT queue right after each FEPI. A short run of dummy matmuls on the
zero tile warms the PE p-state ramp while the first x DMA flies.
"""

import sys
from contextlib import ExitStack

import numpy as np

sys.path.insert(0, "/opt/trn_rl_repo")

import ml_dtypes  # noqa: E402

import concourse.bacc as bacc  # noqa: E402
import concourse.dve_ops as dve_ops  # noqa: E402
import concourse.tile as tile  # noqa: E402
from concourse import mybir  # noqa: E402
from concourse.bass_utils import run_bass_kernel_spmd  # noqa: E402
from concourse.dve_spec import (  # noqa: E402
    C0 as DC0,
    C1 as DC1,
    C2 as DC2,
    One as DOne,
    Spec,
    Src0 as DSrc0,
    Src1 as DSrc1,
    _has_src1,
    lower as dve_lower,
    maxx,
    minn,
    relu as drelu,
)
from concourse.dve_uop import DveOpSpec  # noqa: E402
from concourse.dve_table_gen import dve_ver_for  # noqa: E402
from concourse.dve_ops import DveOp  # noqa: E402

F32 = mybir.dt.float32
F16 = mybir.dt.float16
BF16 = mybir.dt.bfloat16
ALU = mybir.AluOpType
AFT = mybir.ActivationFunctionType
BF16NP = ml_dtypes.bfloat16

C_MAGIC = float(np.float32(12582912.0))  # 1.5 * 2**23

N_CORES = 8
B_LOC = 8  # images per core
PAIRS = B_LOC // 2


def _register_dve_op(name, spec, subdim=False):
    """Register a custom DVE op at runtime (table is generated per-NEFF)."""
    for o in dve_ops.OPS:
        if o.name == name:
            return o
    row = dve_ops._CUSTOM_DVE_ROW_BASE + len(dve_ops.OPS)
    assert row < 0x20
    shas = {}
    for ver in ("v3", "v4"):
        tmp = DveOpSpec(
            name=name, opcode=row, uops=dve_lower(spec, ver=ver),
            rd1_en=_has_src1(spec),
        )
        shas[ver] = tmp.sha(ver)
    op = DveOp(name, spec, subdim=subdim, uops_sha=shas)
    dve_ops.OPS.append(op)
    dve_ops._SUB_OPCODE_FOR_NAME[name] = row
    dve_ops.CUSTOM_DVE_SPECS[name] = spec
    return op


def _b(in0, in1):
    """Sim helper: in1 streams elementwise on HW; align shapes for numpy."""
    if isinstance(in1, np.ndarray) and in1.size == in0.size:
        return in1.reshape(in0.shape)
    return in1


# quantize epilogue: v = in0*s0 + in1 ; out = clip(rne(v),0,127) + 128
# rne via magic-add: u = v + C rounds to the integer grid (f32 ulp==1 in
# [2^23, 2^24)); clip in the shifted domain [C, C+127]; subtract C-128.
QEPI = _register_dve_op(
    "BNECK_QEPI_ANT",
    Spec(
        body=(minn(maxx((DSrc0 * DC0 + DSrc1) + DC1, DC1), DC1 + DC2) - DC1)
        + (DC2 + DOne),
        reference=lambda in0, in1, s0, s1, imm2: np.minimum(
            np.maximum(np.round(in0 * s0 + _b(in0, in1)), 0.0), imm2
        )
        + (imm2 + 1.0),
    ),
)


# final epilogue: out = min(relu(in0*s0 + in1), s1)
FEPI = _register_dve_op(
    "BNECK_FEPI_ANT",
    Spec(
        body=minn(drelu(DSrc0 * DC0 + DSrc1), DC1),
        reference=lambda in0, in1, s0, s1, imm2: np.minimum(
            np.maximum(in0 * s0 + _b(in0, in1), 0.0), s1
        ),
    ),
)


def _build_nc(pairs=PAIRS):
    nc = bacc.Bacc("TRN2", target_bir_lowering=False, debug=False)

    # x codes, plain int values in [-127,127], layout [pair, q, k, (i h)]
    # per (q,k): 1568 full-res codes (i*784+h) then 392 ee-plane codes
    # (i*196 + stride-2 pixels) so the stride-2 shortcut conv reads a
    # contiguous rhs (strided rhs costs ~30-40ns per matmul on the PE).
    x_d = nc.dram_tensor("x", [pairs, 128, 4, 1960], BF16,
                         kind="ExternalInput")
    # weights pre-transposed to partition-major ON THE HOST: device DMAs
    # are then straight 2D copies (128 big descriptors) — on-device
    # rearranges cost 7-11us of descriptor generation on the DGE
    w1_d = nc.dram_tensor("w1l", [128, 4, 256], BF16, kind="ExternalInput")
    w2_d = nc.dram_tensor("w2l", [128, 9, 2, 256], BF16, kind="ExternalInput")
    w3_d = nc.dram_tensor("w3l", [128, 2, 1024], BF16, kind="ExternalInput")
    ws_d = nc.dram_tensor("wsl", [128, 4, 1024], BF16, kind="ExternalInput")
    b1_d = nc.dram_tensor("beta1", [128, 2], F32, kind="ExternalInput")
    b2_d = nc.dram_tensor("beta2", [128, 2], F32, kind="ExternalInput")
    dl_d = nc.dram_tensor("delta", [128, 8], F32, kind="ExternalInput")
    # packed [pair, m-pair, q, (m2 i h)] to keep the out-DMA 2D; host unpacks
    y_d = nc.dram_tensor("y", [pairs, 4, 128, 784], BF16, kind="ExternalOutput")

    a1, a2, g3 = _SCALES

    with tile.TileContext(nc) as tc, ExitStack() as ctx:
        wp = ctx.enter_context(tc.tile_pool(name="w", bufs=1))
        xqp = ctx.enter_context(tc.tile_pool(name="xq", bufs=3))
        p2p = ctx.enter_context(tc.tile_pool(name="p2", bufs=2))
        t3p = ctx.enter_context(tc.tile_pool(name="t3", bufs=2))
        yop = ctx.enter_context(tc.tile_pool(name="yo", bufs=6))
        pc1 = ctx.enter_context(tc.tile_pool(name="pc1", bufs=3, space="PSUM"))
        pc2 = ctx.enter_context(tc.tile_pool(name="pc2", bufs=2, space="PSUM"))
        pc3 = ctx.enter_context(tc.tile_pool(name="pc3", bufs=3, space="PSUM"))

        # ---- weights + biases ----
        # startup critical path: w1 is the very first ACT-queue DMA while
        # pair-0's first x chunk is the first SP DMA — the two sequencers
        # issue in parallel (each issue costs ~0.65us of sequencer time,
        # so queue assignment matters as much as transfer time). Bias
        # broadcasts (ACT compute) then naturally delay the bulk-weight
        # transfers past the startup x-bandwidth crunch. x pairs + nothing
        # else on SP, y stores on ACT, tiny biases on the GpSimd SWDGE
        # queue.
        # w1 k-slices interleave with the x chunks across both queues in
        # joint consumption order: k0 leads SP, k1-2 lead ACT (so the ACT
        # queue's x-k1 transfer isn't stuck behind all of w1), k3 rides SP
        # after x-k2. (SWDGE delivery tried and measured +7us; whole-w1
        # ahead of x-k1 on ACT measured a 3.7us conv1 stall at k1.)
        w1t = wp.tile([128, 4, 256], BF16, tag="w1t")
        nc.sync.dma_start(w1t[:, 0:1], w1_d[:, 0:1])
        nc.scalar.dma_start(w1t[:, 1:3], w1_d[:, 1:3])

        beta1 = wp.tile([128, 2], F32, tag="beta1")
        nc.gpsimd.dma_start(beta1[:], b1_d[:])
        beta2 = wp.tile([128, 2], F32, tag="beta2")
        nc.gpsimd.dma_start(beta2[:], b2_d[:])
        delta = wp.tile([128, 8], F32, tag="delta")
        nc.gpsimd.dma_start(delta[:], dl_d[:])

        # zero tile: ACT broadcast source + PE warmup operands
        rz = wp.tile([128, 392], BF16, tag="rz")
        nc.vector.memset(rz[:], 0.0)
        # warm the PE p-state ramp with dummy matmuls on the zero tile
        # (~3us of continuous PE work; real data lands at ~9.5us)
        pz = pc1.tile([128, 392], F32, tag="ps1", name="pz")
        for _ in range(10):
            nc.tensor.matmul(pz[:], rz[:, :128], rz[:], start=True, stop=True)
        # preload the ACT function table while the first DMAs fly
        warm_act = wp.tile([128, 1], F32, tag="warm_act")
        nc.scalar.activation(warm_act[:], rz[:, :1], AFT.Relu, bias=0.0, scale=1.0)

        st = {}  # per-pair tiles: xq, p2, t3

        def xq_alloc(p):
            xq = xqp.tile([128, 4, 1960], BF16, tag="xq", name=f"xq_{p}")
            st[p] = {"xq": xq}
            return xq

        def xpiece(eng, p, ks, lo, hi):
            """Load x_d[p][:, ks, lo:hi] on the given queue engine."""
            xq = st[p]["xq"]
            eng.dma_start(xq[:, ks, lo:hi], x_d[p][:, ks, lo:hi])

        def emit_xload(p):
            xq = xq_alloc(p)
            nc.sync.dma_start(xq[:], x_d[p])

        def emit_conv1(p, mid_cb=None):
            xq = st[p]["xq"]
            p2 = []
            for m in range(2):
                # p2 holds conv1 output codes+128 in COLUMN-PARITY layout
                # [pc, i, r(30), x(16)]: plane pc holds padded columns
                # c = 2x+pc, so conv2's stride-2 taps read inner-contiguous
                # rhs; rows padded to 30 PER IMAGE so one rank-3 slice
                # ky:ky+58:2 covers both images (29 rows incl 1 discarded
                # junk row) and conv2 runs N=406 matmuls instead of rank-4
                # rhs. pad value 128 == quantized zero; padded row 0 (both
                # images), the junk row 29, and padded column c=1 (pc=1,
                # x=0) get memsets.
                t = p2p.tile([128, 1856], BF16, tag=f"p2_{m}")
                pv = t.rearrange(
                    "q (pc i r x) -> q pc i r x", pc=2, i=2, r=29, x=16
                )
                nc.gpsimd.memset(pv[:, :, :, 0, :], 128.0)
                nc.gpsimd.memset(pv[:, 1, :, :, 0], 128.0)
                p2.append(t)
            for i in (0, 1):
                ps = {}
                # k-outer across the four (m, hf) quads; three PSUM tiles
                # from pc1, the fourth borrows a pc3 bank
                quads = [(0, 0), (0, 1), (1, 0), (1, 1)]
                for k in range(4):
                    for m, hf in quads:
                        if k == 0:
                            pool = pc3 if (m, hf) == (1, 1) else pc1
                            ps[(m, hf)] = pool.tile(
                                [128, 392], F32,
                                name=f"ps1_{m}_{hf}",
                                tag="ps3" if (m, hf) == (1, 1) else "ps1",
                            )
                        nc.tensor.matmul(
                            ps[(m, hf)][:],
                            w1t[:, k, m * 128 : (m + 1) * 128],
                            xq[:, k, i * 784 + hf * 392 : i * 784 + hf * 392 + 392],
                            start=(k == 0),
                            stop=(k == 3),
                            skip_group_check=True,
                        )
                for m in (0, 1):
                    pv = p2[m].rearrange(
                        "q (pc i r x) -> q pc i r x", pc=2, i=2, r=29, x=16
                    )
                    for hf in (0, 1):
                        psv = ps[(m, hf)].rearrange("q (j e) -> q j e", j=14)
                        for pc in (0, 1):
                            # fused epilogue: codes+128 via exact magic-add
                            # RNE; output cols c=2+e land at (pc=c&1, x=c//2)
                            nc.vector._custom_dve(
                                QEPI,
                                out=pv[
                                    :, pc, i, 1 + 14 * hf : 15 + 14 * hf, 1:15
                                ],
                                in0=psv[:, :, pc::2],
                                in1=bc1[m][:, :196],
                                s0=a1, s1=C_MAGIC, imm2=127.0,
                            )
                if i == 0 and mid_cb is not None:
                    mid_cb()
            st[p]["p2"] = p2

        def emit_conv2(p):
            p2 = st[p]["p2"]
            t3 = []
            for m in range(2):
                ps2 = pc2.tile([128, 392], F32)
                first = True
                for k in range(2):
                    pv = p2[k].rearrange(
                        "q (pc i r x) -> q pc i r x", pc=2, i=2, r=29, x=16
                    )
                    for tp in range(9):
                        ky, kx = divmod(tp, 3)
                        # tap (ky,kx) reads padded cols c=1+kx+2ox -> plane
                        # pc=(1+kx)&1, x=(1+kx)//2 .. +14; rows ky..+26
                        # step 2 (inner dim contiguous)
                        nc.tensor.matmul(
                            ps2[:],
                            w2t[:, tp, k, m * 128 : (m + 1) * 128],
                            pv[
                                :, (1 + kx) & 1, :, ky : min(ky + 28, 29) : 2,
                                (1 + kx) // 2 : (1 + kx) // 2 + 14,
                            ],
                            start=first,
                            stop=(k == 1 and tp == 8),
                        )
                        first = False
                t3m = t3p.tile([128, 392], BF16, tag=f"t3_{m}")
                nc.vector._custom_dve(
                    QEPI, out=t3m[:], in0=ps2[:], in1=bc2[m][:],
                    s0=a2, s1=C_MAGIC, imm2=127.0,
                )
                t3.append(t3m)
            st[p]["t3"] = t3

        def emit_conv3(p, last=False):
            xq, t3 = st[p]["xq"], st[p]["t3"]
            for m in range(8):
                ps3 = pc3.tile([128, 392], F32)
                # shortcut first: only needs xq (contiguous host-shipped
                # ee plane), giving t3's epilogue time
                for k in range(4):
                    nc.tensor.matmul(
                        ps3[:],
                        wst[:, k, m * 128 : (m + 1) * 128],
                        xq[:, k, 1568:1960],
                        start=(k == 0),
                        stop=False,
                        skip_group_check=True,
                    )
                for k in range(2):
                    nc.tensor.matmul(
                        ps3[:],
                        w3t[:, k, m * 128 : (m + 1) * 128],
                        t3[k][:],
                        start=False,
                        stop=(k == 1),
                        skip_group_check=True,
                    )
                # single fused DVE epilogue + per-m store keeps the chain
                # after each m's last matmul short and GpSimd-free
                yo = yop.tile([128, 392], BF16, tag="yot", name="yot")
                nc.vector._custom_dve(
                    FEPI, out=yo[:], in0=ps3[:], in1=dlf[m][:],
                    s0=g3, s1=6.0,
                )
                # last pair: alternate store queues so the final store's
                # issue isn't serialized behind the previous one
                seng = nc.sync if (last and m % 2 == 1) else nc.scalar
                seng.dma_start(
                    y_d[p, m // 2][:, (m % 2) * 392 : (m % 2) * 392 + 392],
                    yo[:],
                )
            del st[p]

        # pair 0+1 x chunks interleaved across BOTH HWDGE queues (one queue
        # sustains ~144 B/ns; conv1 eats ~300). conv1 consumes k in the
        # order 0,2,1,3 to match arrival (SP chunks land before ACT ones).
        xq_alloc(0)
        xq_alloc(1)
        xpiece(nc.sync, 0, 0, 0, 784)       # SP:  p0 k0 i0
        xpiece(nc.scalar, 0, 1, 0, 784)     # ACT: p0 k1 i0
        xpiece(nc.sync, 0, 2, 0, 784)       # SP:  p0 k2 i0
        nc.sync.dma_start(w1t[:, 3:4], w1_d[:, 3:4])  # SP: w1 k3
        xpiece(nc.scalar, 0, 3, 0, 784)     # ACT: p0 k3 i0
        bc1, bc2, dlf = [], [], []
        for m in range(2):
            t = wp.tile([128, 392], F32, tag=f"bc1_{m}", name=f"bc1_{m}")
            nc.scalar.activation(
                t[:], rz[:], AFT.Identity, bias=beta1[:, m : m + 1], scale=0.0
            )
            bc1.append(t)
        # p0's i1 block as two 2-k pieces: finer (k-granular) splits add 2
        # DMA issues and re-trigger the sem-pool issue serialization
        # (measured +19us when tried)
        xpiece(nc.sync, 0, slice(0, 2), 784, 1568)    # SP:  p0 i1 k01
        xpiece(nc.scalar, 0, slice(2, 4), 784, 1568)  # ACT: p0 i1 k23
        xpiece(nc.sync, 1, slice(0, 2), 0, 784)       # SP:  p1 i0 k01
        xpiece(nc.scalar, 1, slice(2, 4), 0, 784)     # ACT: p1 i0 k23
        xpiece(nc.sync, 1, slice(0, 2), 784, 1568)    # SP:  p1 i1 k01
        xpiece(nc.scalar, 1, slice(2, 4), 784, 1568)  # ACT: p1 i1 k23
        w2t = wp.tile([128, 9, 2, 256], BF16, tag="w2t")
        nc.scalar.dma_start(w2t[:], w2_d[:])
        for m in range(2):
            t = wp.tile([128, 392], F32, tag=f"bc2_{m}", name=f"bc2_{m}")
            nc.scalar.activation(
                t[:], rz[:], AFT.Identity, bias=beta2[:, m : m + 1], scale=0.0
            )
            bc2.append(t)
        xpiece(nc.sync, 0, slice(0, 4), 1568, 1960)   # SP: p0 ee planes
        xpiece(nc.sync, 1, slice(0, 4), 1568, 1960)   # SP: p1 ee planes
        wst = wp.tile([128, 4, 1024], BF16, tag="wst")
        nc.scalar.dma_start(wst[:], ws_d[:])
        w3t = wp.tile([128, 2, 1024], BF16, tag="w3t")
        nc.scalar.dma_start(w3t[:], w3_d[:])
        for j in range(8):
            t = wp.tile([128, 392], F32, tag=f"dlf{j}", name=f"dlf{j}")
            nc.scalar.activation(
                t[:], rz[:], AFT.Identity, bias=delta[:, j : j + 1], scale=0.0
            )
            dlf.append(t)

        emit_conv1(0)

        # software pipeline: conv2/conv3 of pair p-1 run under conv1 of pair p
        for p in range(1, pairs):
            emit_conv1(p)
            emit_conv2(p - 1)
            if p + 1 < pairs:
                emit_xload(p + 1)
            emit_conv3(p - 1)
        emit_conv2(pairs - 1)
        emit_conv3(pairs - 1, last=True)
    return nc


_SCALES = (1.0, 1.0, 1.0)


def _prep(w1, b1, w2, b2, w3, b3, wsw, bs):
    """Host-side weight quantization + constant folding (all tiny tensors)."""
    f32 = np.float32

    def qw(w):
        s = f32(np.max(np.abs(w)))
        wq = np.round(np.clip(w / s, f32(-1.0), f32(1.0)) * f32(127.0)).astype(
            np.float32
        )
        return wq, s

    def qb(b):
        return np.round(b * f32(127.0)).astype(np.float32)

    w1q, c1s = qw(w1)  # [256,512,1,1]
    w2q, c2s = qw(w2)  # [256,256,3,3]
    w3q, c3s = qw(w3)  # [1024,256,1,1]
    wsq, css = qw(wsw)  # [1024,512,1,1]
    B1, B2, B3, Bs = qb(b1), qb(b2), qb(b3), qb(bs)

    a1 = f32(2.0) * c1s / f32(127.0)
    a2 = f32(2.0) * c2s / f32(127.0)
    g3 = c3s / f32(2.0 * 16129.0)
    rho = css / c3s

    # lhsT layouts, partition(q)-major so device DMAs are plain 2D copies
    w1l = np.ascontiguousarray(
        w1q[:, :, 0, 0].T.reshape(4, 128, 256).transpose(1, 0, 2).astype(BF16NP)
    )  # [128, 4, 256]
    # w2 taps: [ky,kx] -> lhsT [cin, cout] per tap; -> [128, 9, 2, 256]
    w2l = np.ascontiguousarray(
        w2q.transpose(2, 3, 1, 0)
        .reshape(9, 2, 128, 256)
        .transpose(2, 0, 1, 3)
        .astype(BF16NP)
    )
    w3l = np.ascontiguousarray(
        w3q[:, :, 0, 0].T.reshape(2, 128, 1024).transpose(1, 0, 2).astype(BF16NP)
    )  # [128, 2, 1024]
    ws_sc = (rho * wsq[:, :, 0, 0]).astype(BF16NP)  # [1024,512] scaled bf16
    wsl = np.ascontiguousarray(
        ws_sc.T.reshape(4, 128, 1024).transpose(1, 0, 2)
    )  # [128, 4, 1024]

    # column sums for the +128 code-offset corrections (fp64 exact)
    K2 = w2q.astype(np.float64).sum(axis=(1, 2, 3))  # [256]
    K3 = w3q[:, :, 0, 0].astype(np.float64).sum(axis=1)  # [1024]

    # x codes ship plain (no offset); conv1/conv2 outputs carry +128
    beta1 = (f32(4.0) * B1).astype(np.float32)
    beta2 = (f32(4.0) * B2 - a2 * f32(128.0) * K2.astype(np.float32)).astype(
        np.float32
    )
    delta0 = B3 * c3s / (f32(127.0) * c2s) + Bs / f32(127.0)
    delta = (
        delta0 - (g3.astype(np.float64) * (128.0 * K3)).astype(np.float32)
    ).astype(np.float32)

    beta1 = np.ascontiguousarray(beta1.reshape(2, 128).T)  # [128,2]
    beta2 = np.ascontiguousarray(beta2.reshape(2, 128).T)
    delta = np.ascontiguousarray(delta.reshape(8, 128).T)  # [128,8]

    return dict(
        w1l=w1l, w2l=w2l, w3l=w3l, wsl=wsl,
        beta1=beta1, beta2=beta2, delta=delta,
        a1=float(a1), a2=float(a2), g3=float(g3),
    )


def _quant_x(x):
    """Host fake-quant of x: codes = clip(rne(x/0.5 clipped * 127)) as bf16,
    laid out [core, pair, k, q, (i h)]."""
    f32 = np.float32
    c = np.round(
        np.clip(x.astype(np.float32) / f32(0.5), f32(-1.0), f32(1.0)) * f32(127.0)
    )
    # [64, 512, 28, 28] -> [cores, pairs, 2(i), 4(k), 128(q), 28, 28]
    c = c.reshape(N_CORES, PAIRS, 2, 4, 128, 28, 28)
    # full-res block: [cores, pairs, q, k, i, 784]
    full = c.transpose(0, 1, 4, 3, 2, 5, 6).reshape(
        N_CORES, PAIRS, 128, 4, 1568
    )
    # ee plane (stride-2 pixels) for the shortcut conv: [.., q, k, i*196]
    ee = np.ascontiguousarray(c[..., 0::2, 0::2]).transpose(
        0, 1, 4, 3, 2, 5, 6
    ).reshape(N_CORES, PAIRS, 128, 4, 392)
    out = np.concatenate([full, ee], axis=-1)  # [.., 128, 4, 1960]
    return np.ascontiguousarray(out.astype(BF16NP))


def kernel(x, w1, b1, w2, b2, w3, b3, ws, bs):
    global _SCALES
    xc = _quant_x(np.asarray(x, np.float32))
    pre = _prep(
        np.asarray(w1, np.float32), np.asarray(b1, np.float32),
        np.asarray(w2, np.float32), np.asarray(b2, np.float32),
        np.asarray(w3, np.float32), np.asarray(b3, np.float32),
        np.asarray(ws, np.float32), np.asarray(bs, np.float32),
    )
    _SCALES = (pre["a1"], pre["a2"], pre["g3"])
    nc = _build_nc()
    nc.compile()

    shared = {
        "w1l": pre["w1l"], "w2l": pre["w2l"], "w3l": pre["w3l"],
        "wsl": pre["wsl"], "beta1": pre["beta1"], "beta2": pre["beta2"],
        "delta": pre["delta"],
    }
    in_maps = [{"x": xc[c], **shared} for c in range(N_CORES)]

    import os

    tmpdir = os.environ.get("KERNEL_TRACE_DIR") or None
    if tmpdir:
        os.makedirs(tmpdir, exist_ok=True)
    res = run_bass_kernel_spmd(nc, in_maps, list(range(N_CORES)), tmpdir=tmpdir)
    global LAST_RESULT
    LAST_RESULT = res
    outs = [unpack_y(res.results[c]["y"]) for c in range(N_CORES)]
    return np.ascontiguousarray(np.concatenate(outs, axis=0))


def unpack_y(y):
    """[pairs,4,128,784] packed -> [2*pairs, 1024, 14, 14]."""
    p = y.shape[0]
    y = y.reshape(p, 4, 128, 2, 2, 196)  # (p, mp, q, m2, i, h)
    y = y.transpose(0, 4, 1, 3, 2, 5)  # (p, i, mp, m2, q, h)
    return np.ascontiguousarray(
        y.reshape(2 * p, 1024, 14, 14).astype(np.float32)
    )
